# revision 4
# baseline (speedup 1.0000x reference)
"""Trainium2 Bass kernel for nn_CentersDistance.

logits[k, n] = -||centers[k] - inputs[n]||^2
             = 2*(centers @ inputs.T)[k, n] - ||centers[k]||^2 - ||inputs[n]||^2

Strategy (8 NeuronCores, data-parallel over N):
  * host: transpose both operands so the contraction dim D lands on the SBUF
    partition axis ([D, K] and [D, N] layouts), fold the factor 2 into the
    inputs, precompute the (exact, float64) norm terms.
  * each core: 1024x1024x1024 matmul in bf16 (fp32 PSUM accumulation),
    epilogue on DVE adds -||c||^2 (per-partition scalar) and -||x||^2
    (broadcast row) in a single scalar_tensor_tensor op, store fp32.
"""

import threading

import numpy as np
import ml_dtypes

import concourse.bass as bass
import concourse.mybir as mybir
import concourse.tile as tile
from concourse import bacc
from concourse.bass_utils import run_bass_kernel_spmd

N_CORES = 8
N, K, D = 8192, 1024, 1024
NSH = N // N_CORES  # per-core slab of inputs
P = 128             # SBUF partitions
NF = 512            # matmul moving free dim (one fp32 PSUM bank)

D_TILES = D // P    # 8
M_TILES = K // P    # 8
H_TILES = NSH // NF # 2

_DT = mybir.dt.bfloat16
_NP_DT = ml_dtypes.bfloat16

_cache = threading.local()


def _build_nc():
    nc = bacc.Bacc(
        "TRN2", target_bir_lowering=False, debug=False, num_devices=N_CORES
    )
    ct = nc.dram_tensor("ct", [D, K], _DT, kind="ExternalInput").ap()
    xt = nc.dram_tensor("xt", [D, NSH], _DT, kind="ExternalInput").ap()
    ncsq = nc.dram_tensor(
        "ncsq", [P, M_TILES], mybir.dt.float32, kind="ExternalInput"
    ).ap()
    nxsq = nc.dram_tensor(
        "nxsq", [P, NSH], mybir.dt.float32, kind="ExternalInput"
    ).ap()
    out = nc.dram_tensor("out", [K, NSH], mybir.dt.float32, kind="ExternalOutput").ap()

    ct_r = ct.rearrange("(t p) k -> t p k", p=P)
    xt_r = xt.rearrange("(t p) n -> t p n", p=P)
    out_r = out.rearrange("(m p) n -> m p n", p=P)

    with tile.TileContext(nc) as tc:
        with (
            tc.tile_pool(name="w", bufs=1) as wpool,
            tc.tile_pool(name="c", bufs=1) as cpool,
            tc.tile_pool(name="o", bufs=4) as opool,
            tc.tile_pool(name="ps", bufs=7, space="PSUM") as pspool,
            tc.tile_pool(name="wu", bufs=1, space="PSUM") as wupool,
        ):
            # PE warm-up: ~dummy matmuls on a zeroed tile, no data deps, so
            # the tensor engine is busy during the load phase and the HAM
            # clock gate is fully open (2.4 GHz) when the real matmuls start.
            wu_sb = cpool.tile([P, NF], _DT, tag="wu_sb")
            nc.gpsimd.memset(wu_sb[:], 0.0)
            wu_ps = wupool.tile([P, NF], mybir.dt.float32, tag="wu_ps")
            for _ in range(12):
                nc.tensor.matmul(
                    wu_ps[:], wu_sb[:, 0:P], wu_sb[:], start=True, stop=True
                )

            ct_sb = []
            xt_sb = []
            for d in range(D_TILES):
                t = wpool.tile([P, K], _DT, tag=f"ct{d}")
                nc.sync.dma_start(t[:], ct_r[d])
                ct_sb.append(t)
                t = wpool.tile([P, NSH], _DT, tag=f"xt{d}")
                nc.sync.dma_start(t[:], xt_r[d])
                xt_sb.append(t)
                if d == 1:
                    # epilogue constants — needed much later than the first
                    # ct/xt tiles, so don't put them ahead in the DMA queue
                    ncsq_sb = cpool.tile([P, M_TILES], mybir.dt.float32, tag="ncsq")
                    nc.sync.dma_start(ncsq_sb[:], ncsq)
                    nxsq_sb = cpool.tile([P, NSH], mybir.dt.float32, tag="nxsq")
                    nc.sync.dma_start(nxsq_sb[:], nxsq)

            def epilogue(m, h, ps):
                ot = opool.tile([P, NF], mybir.dt.float32, tag="ot", name=f"ot{m}_{h}")
                nc.vector.scalar_tensor_tensor(
                    ot[:],
                    ps[:],
                    ncsq_sb[:, m : m + 1],
                    nxsq_sb[:, h * NF : (h + 1) * NF],
                    op0=mybir.AluOpType.add,
                    op1=mybir.AluOpType.add,
                )
                nc.sync.dma_start(out_r[m][:, h * NF : (h + 1) * NF], ot[:])

            # Pass 1 (m-tiles 0-3): d outermost so the matmuls pace with the
            # streaming ct/xt DMAs; 8 PSUM banks accumulate concurrently.
            ms = range(M_TILES // 2)
            ps = {}
            for m in ms:
                for h in range(H_TILES):
                    ps[(m, h)] = pspool.tile(
                        [P, NF], mybir.dt.float32, tag="ps", name=f"ps_{m}_{h}"
                    )
            for d in range(D_TILES):
                for m in ms:
                    for h in range(H_TILES):
                        nc.tensor.matmul(
                            ps[(m, h)][:],
                            ct_sb[d][:, m * P : (m + 1) * P],
                            xt_sb[d][:, h * NF : (h + 1) * NF],
                            start=(d == 0),
                            stop=(d == D_TILES - 1),
                        )
            for m in ms:
                for h in range(H_TILES):
                    epilogue(m, h, ps[(m, h)])

            # Pass 2 (m-tiles 4-7): everything is resident now, so run d
            # innermost — each (m, h) output retires early and its DVE
            # epilogue + store overlap the remaining matmuls instead of
            # serializing at the kernel tail.
            for m in range(M_TILES // 2, M_TILES):
                for h in range(H_TILES):
                    p2 = pspool.tile(
                        [P, NF], mybir.dt.float32, tag="ps", name=f"ps2_{m}_{h}"
                    )
                    for d in range(D_TILES):
                        nc.tensor.matmul(
                            p2[:],
                            ct_sb[d][:, m * P : (m + 1) * P],
                            xt_sb[d][:, h * NF : (h + 1) * NF],
                            start=(d == 0),
                            stop=(d == D_TILES - 1),
                        )
                    epilogue(m, h, p2)

    nc.compile()
    return nc


def _get_nc():
    if not hasattr(_cache, "nc"):
        _cache.nc = _build_nc()
    return _cache.nc


def kernel(inputs, centers, _trace=False):
    inputs = np.asarray(inputs, dtype=np.float32)
    centers = np.asarray(centers, dtype=np.float32)

    csq = np.sum(centers.astype(np.float64) ** 2, axis=1)
    xsq = np.sum(inputs.astype(np.float64) ** 2, axis=1)

    ct = np.ascontiguousarray(centers.T).astype(_NP_DT)
    xt2 = np.ascontiguousarray((2.0 * inputs).T.astype(_NP_DT))
    ncsq = np.ascontiguousarray(
        (-csq).reshape(M_TILES, P).T.astype(np.float32)
    )

    in_maps = []
    for i in range(N_CORES):
        sl = slice(i * NSH, (i + 1) * NSH)
        in_maps.append(
            {
                "ct": ct,
                "xt": np.ascontiguousarray(xt2[:, sl]),
                "ncsq": ncsq,
                "nxsq": np.ascontiguousarray(
                    np.broadcast_to(-xsq[sl], (P, NSH))
                ).astype(np.float32),
            }
        )

    nc = _get_nc()
    res = run_bass_kernel_spmd(
        nc, in_maps, core_ids=list(range(N_CORES)), trace=_trace
    )
    if _trace:
        kernel.last_results = res
    out = np.concatenate([r["out"] for r in res.results], axis=1)
    return out


# revision 11
# speedup vs baseline: 1.2505x; 1.2505x over previous
"""Trainium2 Bass kernel for nn_CentersDistance.

logits[k, n] = -||centers[k] - inputs[n]||^2
             = 2*(centers @ inputs.T)[k, n] - ||centers[k]||^2 - ||inputs[n]||^2

Strategy (8 NeuronCores, data-parallel over N):
  * host: transpose both operands so the contraction dim D lands on the SBUF
    partition axis ([D, K] and [D, N] layouts), fold the factor 2 into the
    inputs, precompute the (exact, float64) norm terms.
  * each core: 1024x1024x1024 matmul in bf16 (fp32 PSUM accumulation),
    epilogue on DVE adds -||c||^2 (per-partition scalar) and -||x||^2
    (broadcast row) in a single scalar_tensor_tensor op, store fp32.
"""

import threading
from contextlib import ExitStack

import numpy as np
import ml_dtypes

import concourse.bass as bass
import concourse.mybir as mybir
import concourse.tile as tile
from concourse import bacc
from concourse.bass_utils import run_bass_kernel_spmd

N_CORES = 8
N, K, D = 8192, 1024, 1024
NSH = N // N_CORES  # per-core slab of inputs
P = 128             # SBUF partitions
NF = 512            # matmul moving free dim (one fp32 PSUM bank)

D_TILES = D // P    # 8
M_TILES = K // P    # 8
H_TILES = NSH // NF # 2

_DT = mybir.dt.bfloat16
_NP_DT = ml_dtypes.bfloat16

_cache = threading.local()


def _build_nc():
    nc = bacc.Bacc(
        "TRN2", target_bir_lowering=False, debug=False, num_devices=N_CORES
    )
    ct = nc.dram_tensor("ct", [D, K], _DT, kind="ExternalInput").ap()
    xt = nc.dram_tensor("xt", [D, NSH], _DT, kind="ExternalInput").ap()
    ncsq = nc.dram_tensor(
        "ncsq", [P, M_TILES], mybir.dt.float32, kind="ExternalInput"
    ).ap()
    nxsq = nc.dram_tensor(
        "nxsq", [P, NSH], mybir.dt.float32, kind="ExternalInput"
    ).ap()
    out = nc.dram_tensor("out", [K, NSH], mybir.dt.float32, kind="ExternalOutput").ap()

    ct_r = ct.rearrange("(t p) k -> t p k", p=P)
    xt_r = xt.rearrange("(t p) n -> t p n", p=P)
    out_r = out.rearrange("(m p) n -> m p n", p=P)

    with tile.TileContext(nc) as tc:
        with (
            tc.tile_pool(name="w", bufs=1) as wpool,
            tc.tile_pool(name="c", bufs=1) as cpool,
            tc.tile_pool(name="o", bufs=4) as opool,
            tc.tile_pool(name="ps", bufs=7, space="PSUM") as pspool,
            tc.tile_pool(name="wu", bufs=1, space="PSUM") as wupool,
        ):
            # PE warm-up: ~dummy matmuls on a zeroed tile, no data deps, so
            # the tensor engine is busy during the load phase and the HAM
            # clock gate is fully open (2.4 GHz) when the real matmuls start.
            wu_sb = cpool.tile([P, NF], _DT, tag="wu_sb")
            nc.gpsimd.memset(wu_sb[:], 0.0)
            wu_ps = wupool.tile([P, NF], mybir.dt.float32, tag="wu_ps")
            for _ in range(4):
                nc.tensor.matmul(
                    wu_ps[:], wu_sb[:, 0:P], wu_sb[:], start=True, stop=True
                )

            ct_sb = []
            xt_sb = []
            for d in range(D_TILES):
                t = wpool.tile([P, K], _DT, tag=f"ct{d}")
                nc.sync.dma_start(t[:], ct_r[d])
                ct_sb.append(t)
                t = wpool.tile([P, NSH], _DT, tag=f"xt{d}")
                nc.sync.dma_start(t[:], xt_r[d])
                xt_sb.append(t)
                if d == 3:
                    # epilogue constants — needed much later than the ct/xt
                    # tiles; use the GpSimd DMA queue so they don't serialize
                    # with the load stream on the Sync queue
                    ncsq_sb = cpool.tile([P, M_TILES], mybir.dt.float32, tag="ncsq")
                    nc.gpsimd.dma_start(ncsq_sb[:], ncsq)
                    nxsq_sb = cpool.tile([P, NSH], mybir.dt.float32, tag="nxsq")
                    nc.gpsimd.dma_start(nxsq_sb[:], nxsq)

            def epilogue(m, h, ps):
                ot = opool.tile([P, NF], mybir.dt.float32, tag="ot", name=f"ot{m}_{h}")
                nc.vector.scalar_tensor_tensor(
                    ot[:],
                    ps[:],
                    ncsq_sb[:, m : m + 1],
                    nxsq_sb[:, h * NF : (h + 1) * NF],
                    op0=mybir.AluOpType.add,
                    op1=mybir.AluOpType.add,
                )
                nc.sync.dma_start(out_r[m][:, h * NF : (h + 1) * NF], ot[:])

            # Pass 1 (m-tiles 0-3): d outermost so the matmuls pace with the
            # streaming ct/xt DMAs; 8 PSUM banks accumulate concurrently.
            ms = range(M_TILES // 2)
            ps = {}
            for m in ms:
                for h in range(H_TILES):
                    ps[(m, h)] = pspool.tile(
                        [P, NF], mybir.dt.float32, tag="ps", name=f"ps_{m}_{h}"
                    )
            for d in range(D_TILES):
                for m in ms:
                    for h in range(H_TILES):
                        nc.tensor.matmul(
                            ps[(m, h)][:],
                            ct_sb[d][:, m * P : (m + 1) * P],
                            xt_sb[d][:, h * NF : (h + 1) * NF],
                            start=(d == 0),
                            stop=(d == D_TILES - 1),
                        )
            for m in ms:
                for h in range(H_TILES):
                    epilogue(m, h, ps[(m, h)])

            # Pass 2 (m-tiles 4-7): everything is resident now, so run d
            # innermost — each (m, h) output retires early and its DVE
            # epilogue + store overlap the remaining matmuls instead of
            # serializing at the kernel tail.
            for m in range(M_TILES // 2, M_TILES):
                for h in range(H_TILES):
                    p2 = pspool.tile(
                        [P, NF], mybir.dt.float32, tag="ps", name=f"ps2_{m}_{h}"
                    )
                    for d in range(D_TILES):
                        nc.tensor.matmul(
                            p2[:],
                            ct_sb[d][:, m * P : (m + 1) * P],
                            xt_sb[d][:, h * NF : (h + 1) * NF],
                            start=(d == 0),
                            stop=(d == D_TILES - 1),
                        )
                    epilogue(m, h, p2)

    nc.compile()
    return nc


def _build_nc_raw():
    """Raw-Block implementation: same dataflow as the Tile version but with
    5 hand-placed semaphores, because the NEFF epilogue resets every
    declared semaphore serially (~115 ns each) — Tile's ~50 sems cost ~6 us
    of pure tail on every run."""
    nc = bacc.Bacc(
        "TRN2", target_bir_lowering=False, debug=False, num_devices=N_CORES
    )
    ct = nc.dram_tensor("ct", [D, K], _DT, kind="ExternalInput").ap()
    xt = nc.dram_tensor("xt", [D, NSH], _DT, kind="ExternalInput").ap()
    ncsq = nc.dram_tensor(
        "ncsq", [P, M_TILES], mybir.dt.float32, kind="ExternalInput"
    ).ap()
    nxsq = nc.dram_tensor(
        "nxsq", [P, NSH], mybir.dt.float32, kind="ExternalInput"
    ).ap()
    out = nc.dram_tensor("out", [K, NSH], mybir.dt.float32, kind="ExternalOutput").ap()

    ct_r = ct.rearrange("(t p) k -> t p k", p=P)
    xt_r = xt.rearrange("(t p) n -> t p n", p=P)
    out_r = out.rearrange("(m p) n -> m p n", p=P)

    G = M_TILES * H_TILES          # 16 output groups of [128, 512]
    GP1 = G // 2                   # groups 0-7 -> pass 1 (m 0-3)
    N_WU = 6                       # PE warm-up matmuls

    def g_mh(g):
        return g // H_TILES, g % H_TILES

    with (
        nc.sbuf_tensor("wu_sb", [P, NF], _DT) as wu_sb,
        nc.sbuf_tensor("ncsq_sb", [P, M_TILES], mybir.dt.float32) as ncsq_sb,
        nc.sbuf_tensor("nxsq_sb", [P, NSH], mybir.dt.float32) as nxsq_sb,
        nc.sbuf_tensor("ot_sb", [P, G * NF], mybir.dt.float32) as ot_sb,
    ):
        with (
            ExitStack() as stack,
            nc.semaphore() as wu_sem,
            nc.semaphore() as const_sem,
            nc.semaphore() as mm_sem,
            nc.semaphore() as dve_sem,
            nc.semaphore() as dma_out,
            nc.Block() as block,
        ):
            # one sem per d-tile pair: HW-DGE completions of equal-size DMAs
            # are *usually* in issue order, but HBM contention from the other
            # 7 cores can invert them — a shared counter would then let the
            # PE read a tile that is not fully written.
            d_sems = [
                stack.enter_context(nc.semaphore(f"d_sem{i}"))
                for i in range(D_TILES)
            ]
            ct_sb = [
                stack.enter_context(nc.sbuf_tensor(f"ct_sb{d}", [P, K], _DT))
                for d in range(D_TILES)
            ]
            xt_sb = [
                stack.enter_context(nc.sbuf_tensor(f"xt_sb{d}", [P, NSH], _DT))
                for d in range(D_TILES)
            ]
            ps = [
                stack.enter_context(
                    nc.psum_tensor(f"ps{b}", [P, NF], mybir.dt.float32)
                )
                for b in range(GP1)
            ]

            @block.sync
            def _(sync):
                for d in range(D_TILES):
                    sync.dma_start(ct_sb[d][:], ct_r[d]).then_inc(d_sems[d], 16)
                    sync.dma_start(xt_sb[d][:], xt_r[d]).then_inc(d_sems[d], 16)
                # consts last: only the DVE epilogue needs them
                sync.dma_start(ncsq_sb[:], ncsq).then_inc(const_sem, 16)
                sync.dma_start(nxsq_sb[:], nxsq).then_inc(const_sem, 16)
                for g in range(G):
                    m, h = g_mh(g)
                    sync.wait_ge(dve_sem, g + 1)
                    sync.dma_start(
                        out_r[m][:, h * NF : (h + 1) * NF],
                        ot_sb[:, g * NF : (g + 1) * NF],
                    ).then_inc(dma_out, 16)
                sync.wait_ge(dma_out, G * 16)

            @block.tensor
            def _(tensor):
                # warm-up: open the HAM clock gate while the loads stream
                tensor.wait_ge(wu_sem, 1)
                for _ in range(N_WU):
                    nc.tensor.matmul(
                        ps[GP1 - 1][:], wu_sb[:, 0:P], wu_sb[:], start=True, stop=True
                    )
                # pass 1: groups 0-7 accumulate in banks 0-7, d outermost so
                # matmuls pace with the streaming loads
                for d in range(D_TILES):
                    tensor.wait_ge(d_sems[d], 32)
                    for g in range(GP1):
                        m, h = g_mh(g)
                        mm = nc.tensor.matmul(
                            ps[g][:],
                            ct_sb[d][:, m * P : (m + 1) * P],
                            xt_sb[d][:, h * NF : (h + 1) * NF],
                            start=(d == 0),
                            stop=(d == D_TILES - 1),
                        )
                        if d == D_TILES - 1:
                            mm.then_inc(mm_sem, 1)
                # pass 2: groups 8-15 reuse banks 0-7 once the DVE epilogue
                # has drained the pass-1 group from that bank
                for g in range(GP1, G):
                    m, h = g_mh(g)
                    tensor.wait_ge(dve_sem, g - GP1 + 1)
                    for d in range(D_TILES):
                        mm = nc.tensor.matmul(
                            ps[g - GP1][:],
                            ct_sb[d][:, m * P : (m + 1) * P],
                            xt_sb[d][:, h * NF : (h + 1) * NF],
                            start=(d == 0),
                            stop=(d == D_TILES - 1),
                        )
                    mm.then_inc(mm_sem, 1)

            @block.vector
            def _(vector):
                nc.vector.memset(wu_sb[:], 0.0).then_inc(wu_sem, 1)
                vector.wait_ge(const_sem, 32)  # consts present
                for g in range(G):
                    m, h = g_mh(g)
                    vector.wait_ge(mm_sem, g + 1)
                    nc.vector.scalar_tensor_tensor(
                        ot_sb[:, g * NF : (g + 1) * NF],
                        ps[g % GP1][:],
                        ncsq_sb[:, m : m + 1],
                        nxsq_sb[:, h * NF : (h + 1) * NF],
                        op0=mybir.AluOpType.add,
                        op1=mybir.AluOpType.add,
                    ).then_inc(dve_sem, 1)

    nc.compile()
    return nc


def _get_nc():
    if not hasattr(_cache, "nc"):
        _cache.nc = _build_nc_raw()
    return _cache.nc


def kernel(inputs, centers, _trace=False):
    inputs = np.asarray(inputs, dtype=np.float32)
    centers = np.asarray(centers, dtype=np.float32)

    csq = np.sum(centers.astype(np.float64) ** 2, axis=1)
    xsq = np.sum(inputs.astype(np.float64) ** 2, axis=1)

    ct = np.ascontiguousarray(centers.T).astype(_NP_DT)
    xt2 = np.ascontiguousarray((2.0 * inputs).T.astype(_NP_DT))
    ncsq = np.ascontiguousarray(
        (-csq).reshape(M_TILES, P).T.astype(np.float32)
    )

    in_maps = []
    for i in range(N_CORES):
        sl = slice(i * NSH, (i + 1) * NSH)
        in_maps.append(
            {
                "ct": ct,
                "xt": np.ascontiguousarray(xt2[:, sl]),
                "ncsq": ncsq,
                "nxsq": np.ascontiguousarray(
                    np.broadcast_to(-xsq[sl], (P, NSH))
                ).astype(np.float32),
            }
        )

    nc = _get_nc()
    res = run_bass_kernel_spmd(
        nc, in_maps, core_ids=list(range(N_CORES)), trace=_trace
    )
    if _trace:
        kernel.last_results = res
    out = np.concatenate([r["out"] for r in res.results], axis=1)
    return out
